# revision 39
# baseline (speedup 1.0000x reference)
"""GQA causal attention block (B=2, L=2048, d_model=2048, 32 Q heads / 8 KV heads)
on 8 TRN2 NeuronCores.

Sharding: 8-way tensor parallel over heads, batch-paired. Core c owns q-heads
[4c, 4c+4) and kv-head c FOR BOTH batches.

Layouts:
  - qT (bf16): 4 tiles [128, L] = head-pair x batch: tile (t, b) holds heads
    2t (partitions 0:64) and 2t+1 (64:128) of batch b, feature-major.
  - kT (bf16): per batch a [128, L] tile with the kv head DUPLICATED in both
    partition halves, so scores for odd heads read lhsT/rhs at matching base 64.
  - v_aug (bf16): per key-block [128, 130]: cols 0:65 = batch0 (v | ones),
    65:130 = batch1 (v | ones).

Per head+batch, causal attention runs in transposed layout: scores^T = matmul
(kT stationary, qT moving), exp on ScalarE straight out of PSUM into bf16 P
tiles (unnormalized), AV matmul against V-with-ones-column emits both attn^T
and the softmax denominator. One 8-core AllToAll switches head-sharding ->
sequence-sharding; each core then normalizes (reciprocal + partition-broadcast
DMA + DVE multiply) and runs o_proj (fp32r) against the full Wo for its 512
output rows. The host just stacks rows.
"""

import os
import sys
import math

os.environ.setdefault("MYCRO_LOCAL_CACHE", "1")
for _p in ("/opt/trn_rl_repo",):
    if os.path.isdir(_p) and _p not in sys.path:
        sys.path.insert(0, _p)

import numpy as np

import concourse.bass as bass
import concourse.bacc as bacc
import concourse.mybir as mybir
import concourse.tile as tile
from concourse.bass_utils import run_bass_kernel_spmd
from concourse.masks import make_identity

F32 = mybir.dt.float32
F32R = mybir.dt.float32r
BF16 = mybir.dt.bfloat16
# attention operand dtype: f32r (fp32 "HIGH" matmul mode) runs unthrottled on
# the PE, while bf16 trips the activity governor's 50% utilization cap and
# ends up slower per row.
ATT = mybir.dt.float32r
Exp = mybir.ActivationFunctionType.Exp

D = 2048          # d_model
L = 2048          # sequence length
DH = 64           # head dim
B = 2             # batch
NCORES = 8
NH_L = 4          # local q heads per core (per batch)
QF = NH_L * DH    # 256 local q features per batch
LC1 = 256         # phase-1 l-chunk (moving dim)
NLC1 = L // LC1   # 8
LC = 512          # attention l-tile
NLC = L // LC     # 4
NB = L // 128     # 16 key blocks of 128
SH = DH           # 64 rows per per-head A2A chunk shard (normalized attn rows)
SCALE = 1.0 / math.sqrt(DH)

_CACHE = {}


def _mmr(nc, out, lhsT, rhs, **kw):
    """float32r matmul (TF32-ish). Operands must come from f32r-producing
    instructions (gpsimd casting DMA / DVE ops)."""
    nc.tensor.matmul(out, lhsT, rhs, **kw)


def _build_nc():
    nc = bacc.Bacc(
        "TRN2",
        target_bir_lowering=False,
        debug=False,
        enable_asserts=False,
        num_devices=NCORES,
    )
    xT0 = nc.dram_tensor("xT0", [D, L], F32, kind="ExternalInput")
    xT1 = nc.dram_tensor("xT1", [D, L], F32, kind="ExternalInput")
    wqT = nc.dram_tensor("wqT", [D, QF], F32, kind="ExternalInput")
    wkT = nc.dram_tensor("wkT", [D, DH], F32, kind="ExternalInput")
    wvT = nc.dram_tensor("wvT", [D, DH], F32, kind="ExternalInput")
    woT = nc.dram_tensor("woT", [D, D], F32, kind="ExternalInput")
    y = nc.dram_tensor("y", [LC, D], F32, kind="ExternalOutput")

    with tile.TileContext(nc) as tc:
        with tc.tile_pool(name="dram", bufs=1, space="DRAM") as dram:
            # one A2A bounce pair per local head so each chunk's collective
            # can fire as soon as that head's attention finishes
            bins = [
                dram.tile([NCORES * SH, LC], F32, name=f"bounce_in{j}")
                for j in range(NH_L)
            ]
            bouts = [
                dram.tile([NCORES * SH, LC], F32, name=f"bounce_out{j}")
                for j in range(NH_L)
            ]
            rscr = dram.tile([16, 1024], F32, name="rscr")
            with tc.tile_pool(name="const", bufs=1) as const:
                ident = const.tile([128, 128], F32, name="ident")
                make_identity(nc, ident)

                with tc.tile_pool(name="wo", bufs=2) as wop:
                    with tc.tile_pool(name="pers", bufs=1) as pers:
                        # q: [pair t][batch b] -> [128, L] (heads 2t | 2t+1)
                        qT = [
                            [
                                pers.tile([128, L], ATT, name=f"qT{t}{b}")
                                for b in range(2)
                            ]
                            for t in range(2)
                        ]
                        # kT per batch, kv head duplicated in both halves
                        kT = [
                            pers.tile([128, L], ATT, name=f"kT{b}") for b in range(2)
                        ]
                        vaug = pers.tile([128, NB * 130], ATT, name="vaug")
                        va = vaug.rearrange("p (b c) -> p b c", c=130)
                        nc.gpsimd.memset(va[:, :, 64:65].bitcast(F32), 1.0)
                        nc.gpsimd.memset(va[:, :, 129:130].bitcast(F32), 1.0)

                        _phase1_qkv(
                            nc, tc, xT0, xT1, wqT, wkT, wvT, qT, kT, va, ident
                        )
                        # prefetch the first even-half Wo tile; queued after
                        # phase 1's DMAs, it lands during attention
                        wo_pre = {0: _wo_load(nc, wop, woT, 0, 0)}
                        _phase2_attn(nc, tc, qT, kT, va, bins, bouts, rscr)
                    # pers closed: qT/kT/vaug SBUF is recycled for o_proj
                    _phase4_oproj(nc, tc, bouts, woT, wop, wo_pre, y)
    nc.finalize()  # bacc: register allocation, ACT table loads, etc.
    return nc


def _phase1_qkv(nc, tc, xT0, xT1, wqT, wkT, wvT, qT, kT, va, ident):
    """Projections. q: two M=128 f32r matmuls per (batch, db). k|v packed
    into ONE M=128 stationary per batch (k feats in PE cols 0:64 -> psum
    partitions 0:64, v in 64:128), saving a quarter of the phase-1 matmul
    rows + LDWEIGHTS. The partition-64 dup half of kT is filled via a
    staging tile + SBUF->SBUF DMA (matmul psum dst must start at
    partition 0)."""
    with (
        tc.tile_pool(name="w1", bufs=1) as wpool,
        tc.tile_pool(name="xc", bufs=2) as xpool,
        tc.tile_pool(name="vt", bufs=2) as vtpool,
        tc.tile_pool(name="p1", bufs=1, space="PSUM") as p1,
    ):
        wq_sb = wpool.tile([128, 16 * QF], F32R, name="wq_sb")
        wkv_sb = wpool.tile([128, 16 * 128], F32R, name="wkv_sb")
        nc.gpsimd.dma_start(
            wq_sb.rearrange("p (b f) -> p b f", f=QF),
            wqT.rearrange("(b p) f -> p b f", p=128),
        )
        wkv_r = wkv_sb.rearrange("p (b f) -> p b f", f=128)
        nc.gpsimd.dma_start(
            wkv_r[:, :, 0:DH], wkT.rearrange("(b p) f -> p b f", p=128)
        )
        nc.gpsimd.dma_start(
            wkv_r[:, :, DH:128], wvT.rearrange("(b p) f -> p b f", p=128)
        )

        for lc in range(NLC1):
            x0 = xpool.tile([128, 16 * LC1], F32R, name="x0", tag="x0")
            x1 = xpool.tile([128, 16 * LC1], F32R, name="x1", tag="x1")
            for xt, xdram in ((x0, xT0), (x1, xT1)):
                nc.gpsimd.dma_start(
                    xt.rearrange("p (b l) -> p b l", l=LC1),
                    xdram[:, lc * LC1 : (lc + 1) * LC1].rearrange(
                        "(b p) l -> p b l", p=128
                    ),
                )
            cols = slice(lc * LC1, (lc + 1) * LC1)
            aq = {
                (g, b): p1.tile([128, LC1], F32, name=f"aq{g}{b}", tag=f"aq{g}{b}")
                for g in range(2)
                for b in range(2)
            }
            akv = [
                p1.tile([128, LC1], F32, name=f"akv{b}", tag=f"akv{b}")
                for b in range(2)
            ]
            for db in range(16):
                rx = (
                    x0[:, db * LC1 : (db + 1) * LC1],
                    x1[:, db * LC1 : (db + 1) * LC1],
                )
                st = dict(start=(db == 0), stop=(db == 15))
                for b in range(2):
                    for g in range(2):
                        wjp = wq_sb[:, db * QF + g * 128 : db * QF + (g + 1) * 128]
                        _mmr(nc, aq[(g, b)][:, :], wjp, rx[b], **st)
                    _mmr(
                        nc, akv[b][:, :],
                        wkv_sb[:, db * 128 : (db + 1) * 128], rx[b], **st,
                    )
            for g in range(2):
                for b in range(2):
                    nc.scalar.copy(qT[g][b][:, cols], aq[(g, b)][:, :])
            for b in range(2):
                nc.scalar.copy(kT[b][0:64, cols], akv[b][0:64, :])
                stk = vtpool.tile([64, LC1], ATT, name="stk", tag=f"stk{b}")
                nc.scalar.copy(stk[:, :], akv[b][0:64, :])
                nc.sync.dma_start(kT[b][64:128, cols], stk[:, :])
            vt0 = vtpool.tile([64, LC1], F32, name="vt0", tag="vt0")
            vt1 = vtpool.tile([64, LC1], F32, name="vt1", tag="vt1")
            nc.scalar.copy(vt0[:, :], akv[0][64:128, :])
            nc.scalar.copy(vt1[:, :], akv[1][64:128, :])
            for s in range(LC1 // 128):
                beta = (lc * LC1) // 128 + s
                tp = p1.tile([128, 128], F32, name="tp", tag="tp", bufs=2)
                nc.tensor.matmul(
                    tp[:, 0:64],
                    vt0[:, s * 128 : (s + 1) * 128],
                    ident[0:64, 0:64],
                    is_transpose=True,
                )
                nc.tensor.matmul(
                    tp[:, 64:128],
                    vt1[:, s * 128 : (s + 1) * 128],
                    ident[0:64, 0:64],
                    is_transpose=True,
                    skip_group_check=True,
                )
                nc.scalar.copy(va[:, beta, 0:64], tp[:, 0:64])
                nc.scalar.copy(va[:, beta, 65:129], tp[:, 64:128])


def _wo_load(nc, wop, woT, dc, par):
    """Load the even (par=0) or odd (par=1) d_model-block half of Wo columns
    [dc*512, (dc+1)*512) as a [128, 8*512] f32r stationary set."""
    tag = "wo_e" if par == 0 else "wo_o"
    wo_t = wop.tile([128, 8 * 512], F32R, name=tag, tag=tag)
    nc.gpsimd.dma_start(
        wo_t.rearrange("p (b d) -> p b d", d=512),
        woT[:, dc * 512 : (dc + 1) * 512].rearrange("(b p) d -> p b d", p=128)[
            :, par::2
        ],
    )
    return wo_t


def _phase2_attn(nc, tc, qT, kT, va, bins, bouts, rscr):
    """Transposed-scores causal attention (f32r QK^T and AV). Head-outer loop
    so each head's A2A chunk collective fires while later heads compute.

    P tile layout per (head, tau): full key blocks b < 4*tau at cols
    [b*512, (b+1)*512); the four diagonal blocks packed contiguously from
    DB = 4*tau*512 at offsets [0, 512, 896, 1152] with widths [512, 384,
    256, 128] (only q columns >= the block start are kept), so the 4
    diagonal exps merge into 2 ACT calls. pa/pb ping-pong on tau parity so
    the next iteration's exp can run while this iteration's AV drains."""
    DOFF = (0, 512, 896, 1152)
    with (
        tc.tile_pool(name="p2s", bufs=1, space="PSUM") as scp,
        tc.tile_pool(name="p2o", bufs=1, space="PSUM") as ovp,
        tc.tile_pool(name="pbuf", bufs=1) as pbp,
        tc.tile_pool(name="stg", bufs=1) as stp,
    ):
        for j in range(4):  # local q head
            for tau in range(NLC):
                t, hh = divmod(j, 2)
                po = 64 * hh  # partition base inside the pair tile
                nb = 4 * tau + 4
                DB = 4 * tau * 512
                par = tau % 2
                pcols = 5376 if par == 0 else 7424
                pa = pbp.tile([128, pcols], ATT, name="pa", tag=f"pa{par}")
                pb = pbp.tile([128, pcols], ATT, name="pb", tag=f"pb{par}")
                lcols = slice(tau * LC, (tau + 1) * LC)
                qa = qT[t][0][po : po + 64, lcols]
                qb = qT[t][1][po : po + 64, lcols]

                # full (unmasked) strips, two key-blocks per exp call
                for b0 in range(0, 4 * tau, 2):
                    for q, kTb, P in ((qa, kT[0], pa), (qb, kT[1], pb)):
                        sc = scp.tile(
                            [128, 1024], F32, name="sc", tag="sc", bufs=3
                        )
                        nc.tensor.matmul(
                            sc[:, 0:512],
                            kTb[po : po + 64, b0 * 128 : (b0 + 1) * 128],
                            q,
                        )
                        nc.tensor.matmul(
                            sc[:, 512:1024],
                            kTb[po : po + 64, (b0 + 1) * 128 : (b0 + 2) * 128],
                            q,
                        )
                        nc.scalar.activation(
                            P[:, b0 * 512 : (b0 + 2) * 512], sc[:, 0:1024], Exp
                        )
                # diagonal strips: only q columns >= block start are computed;
                # two merged exps (dj 0+1 -> 896 cols, dj 2+3 -> 384 cols)
                for b_, (kTb, P) in enumerate(((kT[0], pa), (kT[1], pb))):
                    for dpair in ((0, 1), (2, 3)):
                        sc = scp.tile(
                            [128, 1024], F32, name="sc", tag="sc", bufs=3
                        )
                        soff = 0
                        for dj in dpair:
                            beta = 4 * tau + dj
                            nv = 512 - dj * 128
                            qd = qT[t][b_][
                                po : po + 64,
                                tau * LC + dj * 128 : (tau + 1) * LC,
                            ]
                            nc.tensor.matmul(
                                sc[:, soff : soff + nv],
                                kTb[po : po + 64, beta * 128 : (beta + 1) * 128],
                                qd,
                            )
                            soff += nv
                        nc.scalar.activation(
                            P[:, DB + DOFF[dpair[0]] : DB + DOFF[dpair[0]] + soff],
                            sc[:, 0:soff],
                            Exp,
                        )
                    for dj in range(4):
                        dg = P[:, DB + DOFF[dj] : DB + DOFF[dj] + 128]
                        nc.gpsimd.affine_select(
                            out=dg,
                            in_=dg,
                            compare_op=mybir.AluOpType.is_ge,
                            fill=0.0,
                            base=0,
                            pattern=[[1, 128]],
                            channel_multiplier=-1,
                        )
                # AV (+denominator via the ones column of v_aug); diagonal
                # key blocks only accumulate into their valid q suffix
                oa = ovp.tile([128, 512], F32, name="oa", tag="oa")
                ob = ovp.tile([128, 512], F32, name="ob", tag="ob")
                for b in range(nb):
                    dj = b - 4 * tau
                    if dj < 0:
                        lo, pcol = 0, b * 512
                    else:
                        lo, pcol = dj * 128, DB + DOFF[dj]
                    st = dict(start=(b == 0), stop=(b == nb - 1))
                    nc.tensor.matmul(
                        oa[0:65, lo:512], va[:, b, 0:65],
                        pa[:, pcol : pcol + 512 - lo],
                        skip_group_check=True, **st,
                    )
                    nc.tensor.matmul(
                        ob[0:65, lo:512], va[:, b, 65:130],
                        pb[:, pcol : pcol + 512 - lo],
                        skip_group_check=True, **st,
                    )
                # normalize on the sender (softmax denominators come out of
                # the AV ones-column) so the A2A ships ready-to-use rows and
                # o_proj just DMA-loads them. DMA cannot read PSUM, so copy
                # through SBUF first (batch0 on Scalar, batch1 on Vector).
                # dest shard for (batch bb, l-block tau) is 4*bb + tau.
                st1a = stp.tile([64, 512], F32, name="st1a", tag="st1a")
                st1b = stp.tile([64, 512], F32, name="st1b", tag="st1b")
                nc.scalar.copy(st1a[:, :], oa[0:64, :])
                nc.vector.tensor_copy(st1b[:, :], ob[0:64, :])
                ds = stp.tile([128, 1024], F32, name="ds", tag="ds")
                nc.vector.tensor_copy(ds[64:65, 0:512], oa[64:65, :])
                nc.vector.tensor_copy(ds[64:65, 512:1024], ob[64:65, :])
                nc.vector.reciprocal(ds[64:65, :], ds[64:65, :])
                rrow = 4 * j + tau
                nc.sync.dma_start(rscr[rrow : rrow + 1, :], ds[64:65, :])
                dvb0 = stp.tile([64, 512], F32, name="dvb0", tag="dvb0")
                dvb1 = stp.tile([64, 512], F32, name="dvb1", tag="dvb1")
                nc.sync.dma_start(
                    dvb0[:, :],
                    rscr[rrow : rrow + 1, 0:512].partition_broadcast(64),
                )
                nc.sync.dma_start(
                    dvb1[:, :],
                    rscr[rrow : rrow + 1, 512:1024].partition_broadcast(64),
                )
                stn0 = stp.tile([64, 512], F32, name="stn0", tag="stn0")
                stn1 = stp.tile([64, 512], F32, name="stn1", tag="stn1")
                nc.vector.tensor_mul(stn0[:, :], st1a[:, :], dvb0[:, :])
                nc.vector.tensor_mul(stn1[:, :], st1b[:, :], dvb1[:, :])
                for bb, half in ((0, stn0[:, :]), (1, stn1[:, :])):
                    sh = SH * (4 * bb + tau)
                    nc.sync.dma_start(bins[j][sh : sh + 64, :], half)
            # head j fully staged on every core -> overlap its A2A with the
            # attention compute of the remaining heads
            nc.gpsimd.collective_compute(
                "AllToAll",
                mybir.AluOpType.bypass,
                ins=[bins[j].opt()],
                outs=[bouts[j].opt()],
                replica_groups=[list(range(NCORES))],
            )


def _phase4_oproj(nc, tc, bouts, woT, wop, wo_pre, y):
    """o_proj for this core's 512 sequence rows against the full Wo. The A2A
    chunks arrive pre-normalized: chunk j shard c holds global head 4c+j, so
    o_proj contraction tile k (heads 2k, 2k+1) is filled by chunks (0,1) for
    even k and (2,3) for odd k. Two-pass contraction: the even half runs
    while chunks 2/3 are still in flight (spilling partials to SBUF), the
    odd half finishes after chunk 3 lands; DVE adds the halves."""
    with (
        tc.tile_pool(name="an", bufs=1) as anp,
        tc.tile_pool(name="wo2", bufs=2) as wop2,
        tc.tile_pool(name="ysb", bufs=2) as yp,
        tc.tile_pool(name="esb", bufs=1) as esp,
        tc.tile_pool(name="p4y", bufs=4, space="PSUM") as eyp,
    ):
        ans = [anp.tile([128, 512], F32R, name=f"an{k}") for k in range(16)]

        def load_ans(j):
            for c in range(NCORES):
                # head 4c+j -> ans tile k = 2c + j//2, partition half j%2
                k = 2 * c + j // 2
                po = 64 * (j % 2)
                nc.gpsimd.dma_start(
                    ans[k][po : po + 64, :], bouts[j][SH * c : SH * c + 64, :]
                )

        # wo_e dc=1 right away (dc=0 was prefetched mid-attention); the
        # even-pass ans chunks next so the queue never blocks on cc2/cc3
        wo_pre[1] = _wo_load(nc, wop, woT, 1, 0)
        load_ans(0)
        load_ans(1)
        esbs = {}
        for dc in range(4):
            wo_e = wo_pre[dc] if dc in wo_pre else _wo_load(nc, wop, woT, dc, 0)
            for m in range(4):
                yps = eyp.tile([128, 512], F32, name="ypsE", tag="ypsE")
                for i, k in enumerate(range(0, 16, 2)):
                    _mmr(
                        nc, yps[:, :],
                        ans[k][:, m * 128 : (m + 1) * 128],
                        wo_e[:, i * 512 : (i + 1) * 512],
                        start=(i == 0), stop=(i == 7),
                    )
                esb = esp.tile([128, 512], F32, name=f"esb{dc}{m}")
                nc.scalar.copy(esb[:, :], yps[:, :])
                esbs[(dc, m)] = esb
        # prefetch the first two odd-half Wo tiles BEFORE the ans chunk-2/3
        # DMAs: those wait on cc2/cc3 and would block the queue behind them
        wo_os = {}
        for dc in range(2):
            wo_o = wop2.tile([128, 8 * 512], F32R, name="wo_o", tag="wo_o")
            nc.gpsimd.dma_start(
                wo_o.rearrange("p (b d) -> p b d", d=512),
                woT[:, dc * 512 : (dc + 1) * 512].rearrange(
                    "(b p) d -> p b d", p=128
                )[:, 1::2],
            )
            wo_os[dc] = wo_o
        load_ans(2)
        load_ans(3)
        for dc in range(4):
            if dc in wo_os:
                wo_o = wo_os[dc]
            else:
                wo_o = wop2.tile([128, 8 * 512], F32R, name="wo_o", tag="wo_o")
                nc.gpsimd.dma_start(
                    wo_o.rearrange("p (b d) -> p b d", d=512),
                    woT[:, dc * 512 : (dc + 1) * 512].rearrange(
                        "(b p) d -> p b d", p=128
                    )[:, 1::2],
                )
            for m in range(4):
                yps = eyp.tile([128, 512], F32, name="ypsO", tag="ypsO")
                for i, k in enumerate(range(1, 16, 2)):
                    _mmr(
                        nc, yps[:, :],
                        ans[k][:, m * 128 : (m + 1) * 128],
                        wo_o[:, i * 512 : (i + 1) * 512],
                        start=(i == 0), stop=(i == 7),
                    )
                ysb = yp.tile([128, 512], F32, name="ysb", tag="ysb")
                nc.vector.tensor_add(ysb[:, :], yps[:, :], esbs[(dc, m)][:, :])
                nc.sync.dma_start(
                    y[m * 128 : (m + 1) * 128, dc * 512 : (dc + 1) * 512], ysb[:, :]
                )


def _get_nc():
    if "nc" not in _CACHE:
        _CACHE["nc"] = _build_nc()
    return _CACHE["nc"]


LAST_EXEC_NS = None


def kernel(x, Wq, Wk, Wv, Wo):
    global LAST_EXEC_NS
    x = np.asarray(x, dtype=np.float32)
    Wq = np.asarray(Wq, dtype=np.float32)
    Wk = np.asarray(Wk, dtype=np.float32)
    Wv = np.asarray(Wv, dtype=np.float32)
    Wo = np.asarray(Wo, dtype=np.float32)

    xT0 = np.ascontiguousarray(x[0].T)
    xT1 = np.ascontiguousarray(x[1].T)
    woT = np.ascontiguousarray(Wo.T)

    in_maps = []
    for c in range(NCORES):
        wqT_c = np.ascontiguousarray((SCALE * Wq[QF * c : QF * (c + 1), :]).T)
        wkT_c = np.ascontiguousarray(Wk[DH * c : DH * (c + 1), :].T)
        wvT_c = np.ascontiguousarray(Wv[DH * c : DH * (c + 1), :].T)
        in_maps.append(
            {
                "xT0": xT0,
                "xT1": xT1,
                "wqT": wqT_c,
                "wkT": wkT_c,
                "wvT": wvT_c,
                "woT": woT,
            }
        )

    nc = _get_nc()
    res = run_bass_kernel_spmd(nc, in_maps, core_ids=list(range(NCORES)))
    LAST_EXEC_NS = getattr(res, "exec_time_ns", None)

    out = np.empty((B, L, D), dtype=np.float32)
    for c in range(NCORES):
        b, g = divmod(c, 4)
        out[b, 512 * g : 512 * (g + 1), :] = res.results[c]["y"]
    return out



# revision 43
# speedup vs baseline: 1.1006x; 1.1006x over previous
"""GQA causal attention block (B=2, L=2048, d_model=2048, 32 Q heads / 8 KV heads)
on 8 TRN2 NeuronCores.

Sharding: 8-way tensor parallel over heads, batch-paired. Core c owns q-heads
[4c, 4c+4) and kv-head c FOR BOTH batches.

Layouts:
  - qT (bf16): 4 tiles [128, L] = head-pair x batch: tile (t, b) holds heads
    2t (partitions 0:64) and 2t+1 (64:128) of batch b, feature-major.
  - kT (bf16): per batch a [128, L] tile with the kv head DUPLICATED in both
    partition halves, so scores for odd heads read lhsT/rhs at matching base 64.
  - v_aug (bf16): per key-block [128, 130]: cols 0:65 = batch0 (v | ones),
    65:130 = batch1 (v | ones).

Per head+batch, causal attention runs in transposed layout: scores^T = matmul
(kT stationary, qT moving), exp on ScalarE straight out of PSUM into bf16 P
tiles (unnormalized), AV matmul against V-with-ones-column emits both attn^T
and the softmax denominator. One 8-core AllToAll switches head-sharding ->
sequence-sharding; each core then normalizes (reciprocal + partition-broadcast
DMA + DVE multiply) and runs o_proj (fp32r) against the full Wo for its 512
output rows. The host just stacks rows.
"""

import os
import sys
import math

os.environ.setdefault("MYCRO_LOCAL_CACHE", "1")
for _p in ("/opt/trn_rl_repo",):
    if os.path.isdir(_p) and _p not in sys.path:
        sys.path.insert(0, _p)

import numpy as np

import concourse.bass as bass
import concourse.bacc as bacc
import concourse.mybir as mybir
import concourse.tile as tile
from concourse.bass_utils import run_bass_kernel_spmd
from concourse.masks import make_identity

F32 = mybir.dt.float32
F32R = mybir.dt.float32r
BF16 = mybir.dt.bfloat16
# attention operand dtype: f32r (fp32 "HIGH" matmul mode) runs unthrottled on
# the PE, while bf16 trips the activity governor's 50% utilization cap and
# ends up slower per row.
ATT = mybir.dt.float32r
Exp = mybir.ActivationFunctionType.Exp

D = 2048          # d_model
L = 2048          # sequence length
DH = 64           # head dim
B = 2             # batch
NCORES = 8
NH_L = 4          # local q heads per core (per batch)
QF = NH_L * DH    # 256 local q features per batch
LC1 = 256         # phase-1 l-chunk (moving dim)
NLC1 = L // LC1   # 8
LC = 512          # attention l-tile
NLC = L // LC     # 4
NB = L // 128     # 16 key blocks of 128
SH = DH           # 64 rows per per-head A2A chunk shard (normalized attn rows)
SCALE = 1.0 / math.sqrt(DH)

_CACHE = {}


def _mmr(nc, out, lhsT, rhs, **kw):
    """float32r matmul (TF32-ish). Operands must come from f32r-producing
    instructions (gpsimd casting DMA / DVE ops)."""
    nc.tensor.matmul(out, lhsT, rhs, **kw)


def _build_nc():
    nc = bacc.Bacc(
        "TRN2",
        target_bir_lowering=False,
        debug=False,
        enable_asserts=False,
        num_devices=NCORES,
    )
    xT0 = nc.dram_tensor("xT0", [D, L], F32, kind="ExternalInput")
    xT1 = nc.dram_tensor("xT1", [D, L], F32, kind="ExternalInput")
    wqT = nc.dram_tensor("wqT", [D, QF], F32, kind="ExternalInput")
    wkT = nc.dram_tensor("wkT", [D, DH], F32, kind="ExternalInput")
    wvT = nc.dram_tensor("wvT", [D, DH], F32, kind="ExternalInput")
    woT = nc.dram_tensor("woT", [D, D], F32, kind="ExternalInput")
    y = nc.dram_tensor("y", [LC, D], F32, kind="ExternalOutput")

    with tile.TileContext(nc) as tc:
        with tc.tile_pool(name="dram", bufs=1, space="DRAM") as dram:
            # one A2A bounce pair per local head so each chunk's collective
            # can fire as soon as that head's attention finishes
            bins = [
                dram.tile([NCORES * SH, LC], BF16, name=f"bounce_in{j}")
                for j in range(NH_L)
            ]
            bouts = [
                dram.tile([NCORES * SH, LC], BF16, name=f"bounce_out{j}")
                for j in range(NH_L)
            ]
            rscr = dram.tile([16, 1024], F32, name="rscr")
            with tc.tile_pool(name="const", bufs=1) as const:
                ident = const.tile([128, 128], F32, name="ident")
                make_identity(nc, ident)

                with tc.tile_pool(name="wo", bufs=2) as wop, tc.tile_pool(
                    name="an", bufs=1
                ) as anp:
                    ans = [
                        anp.tile([128, 512], BF16, name=f"an{k}")
                        for k in range(16)
                    ]

                    def load_ans(j):
                        for c in range(NCORES):
                            # head 4c+j -> ans tile 2c + j//2, partitions j%2
                            k = 2 * c + j // 2
                            po = 64 * (j % 2)
                            nc.gpsimd.dma_start(
                                ans[k][po : po + 64, :],
                                bouts[j][SH * c : SH * c + 64, :],
                            )

                    with tc.tile_pool(name="pers", bufs=1) as pers:
                        # q: [pair t][batch b] -> [128, L] (heads 2t | 2t+1)
                        qT = [
                            [
                                pers.tile([128, L], ATT, name=f"qT{t}{b}")
                                for b in range(2)
                            ]
                            for t in range(2)
                        ]
                        # kT per batch, kv head duplicated in both halves
                        kT = [
                            pers.tile([128, L], ATT, name=f"kT{b}") for b in range(2)
                        ]
                        vaug = pers.tile([128, NB * 130], ATT, name="vaug")
                        va = vaug.rearrange("p (b c) -> p b c", c=130)
                        nc.gpsimd.memset(va[:, :, 64:65].bitcast(F32), 1.0)
                        nc.gpsimd.memset(va[:, :, 129:130].bitcast(F32), 1.0)

                        _phase1_qkv(
                            nc, tc, xT0, xT1, wqT, wkT, wvT, qT, kT, va, ident
                        )
                        # prefetch the first even-half Wo tile; queued after
                        # phase 1's DMAs, it lands during attention
                        wo_pre = {0: _wo_load(nc, wop, woT, 0, 0)}
                        _phase2_attn(nc, tc, qT, kT, va, bins, bouts, rscr, load_ans)
                    # pers closed: qT/kT/vaug SBUF is recycled for o_proj
                    _phase4_oproj(nc, tc, ans, load_ans, woT, wop, wo_pre, y)
    nc.finalize()  # bacc: register allocation, ACT table loads, etc.
    return nc


def _phase1_qkv(nc, tc, xT0, xT1, wqT, wkT, wvT, qT, kT, va, ident):
    """Projections. q: two M=128 f32r matmuls per (batch, db). k|v packed
    into ONE M=128 stationary per batch (k feats in PE cols 0:64 -> psum
    partitions 0:64, v in 64:128), saving a quarter of the phase-1 matmul
    rows + LDWEIGHTS. The partition-64 dup half of kT is filled via a
    staging tile + SBUF->SBUF DMA (matmul psum dst must start at
    partition 0)."""
    with (
        tc.tile_pool(name="w1", bufs=1) as wpool,
        tc.tile_pool(name="xc", bufs=2) as xpool,
        tc.tile_pool(name="vt", bufs=2) as vtpool,
        tc.tile_pool(name="p1", bufs=1, space="PSUM") as p1,
    ):
        wq_sb = wpool.tile([128, 16 * QF], F32R, name="wq_sb")
        wkv_sb = wpool.tile([128, 16 * 128], F32R, name="wkv_sb")
        nc.gpsimd.dma_start(
            wq_sb.rearrange("p (b f) -> p b f", f=QF),
            wqT.rearrange("(b p) f -> p b f", p=128),
        )
        wkv_r = wkv_sb.rearrange("p (b f) -> p b f", f=128)
        nc.gpsimd.dma_start(
            wkv_r[:, :, 0:DH], wkT.rearrange("(b p) f -> p b f", p=128)
        )
        nc.gpsimd.dma_start(
            wkv_r[:, :, DH:128], wvT.rearrange("(b p) f -> p b f", p=128)
        )

        for lc in range(NLC1):
            x0 = xpool.tile([128, 16 * LC1], F32R, name="x0", tag="x0")
            x1 = xpool.tile([128, 16 * LC1], F32R, name="x1", tag="x1")
            for xt, xdram in ((x0, xT0), (x1, xT1)):
                nc.gpsimd.dma_start(
                    xt.rearrange("p (b l) -> p b l", l=LC1),
                    xdram[:, lc * LC1 : (lc + 1) * LC1].rearrange(
                        "(b p) l -> p b l", p=128
                    ),
                )
            cols = slice(lc * LC1, (lc + 1) * LC1)
            aq = {
                (g, b): p1.tile([128, LC1], F32, name=f"aq{g}{b}", tag=f"aq{g}{b}")
                for g in range(2)
                for b in range(2)
            }
            akv = [
                p1.tile([128, LC1], F32, name=f"akv{b}", tag=f"akv{b}")
                for b in range(2)
            ]
            for db in range(16):
                rx = (
                    x0[:, db * LC1 : (db + 1) * LC1],
                    x1[:, db * LC1 : (db + 1) * LC1],
                )
                st = dict(start=(db == 0), stop=(db == 15))
                for b in range(2):
                    for g in range(2):
                        wjp = wq_sb[:, db * QF + g * 128 : db * QF + (g + 1) * 128]
                        _mmr(nc, aq[(g, b)][:, :], wjp, rx[b], **st)
                    _mmr(
                        nc, akv[b][:, :],
                        wkv_sb[:, db * 128 : (db + 1) * 128], rx[b], **st,
                    )
            for g in range(2):
                for b in range(2):
                    nc.scalar.copy(qT[g][b][:, cols], aq[(g, b)][:, :])
            for b in range(2):
                nc.scalar.copy(kT[b][0:64, cols], akv[b][0:64, :])
                stk = vtpool.tile([64, LC1], ATT, name="stk", tag=f"stk{b}")
                nc.scalar.copy(stk[:, :], akv[b][0:64, :])
                nc.sync.dma_start(kT[b][64:128, cols], stk[:, :])
            vt0 = vtpool.tile([64, LC1], F32, name="vt0", tag="vt0")
            vt1 = vtpool.tile([64, LC1], F32, name="vt1", tag="vt1")
            nc.scalar.copy(vt0[:, :], akv[0][64:128, :])
            nc.scalar.copy(vt1[:, :], akv[1][64:128, :])
            for s in range(LC1 // 128):
                beta = (lc * LC1) // 128 + s
                tp = p1.tile([128, 128], F32, name="tp", tag="tp", bufs=2)
                nc.tensor.matmul(
                    tp[:, 0:64],
                    vt0[:, s * 128 : (s + 1) * 128],
                    ident[0:64, 0:64],
                    is_transpose=True,
                )
                nc.tensor.matmul(
                    tp[:, 64:128],
                    vt1[:, s * 128 : (s + 1) * 128],
                    ident[0:64, 0:64],
                    is_transpose=True,
                    skip_group_check=True,
                )
                nc.scalar.copy(va[:, beta, 0:64], tp[:, 0:64])
                nc.scalar.copy(va[:, beta, 65:129], tp[:, 64:128])


def _wo_load(nc, wop, woT, dc, par):
    """Load the even (par=0) or odd (par=1) d_model-block half of Wo columns
    [dc*512, (dc+1)*512) as a [128, 8*512] f32r stationary set."""
    tag = "wo_e" if par == 0 else "wo_o"
    wo_t = wop.tile([128, 8 * 512], BF16, name=tag, tag=tag)
    nc.gpsimd.dma_start(
        wo_t.rearrange("p (b d) -> p b d", d=512),
        woT[:, dc * 512 : (dc + 1) * 512].rearrange("(b p) d -> p b d", p=128)[
            :, par::2
        ],
    )
    return wo_t


def _phase2_attn(nc, tc, qT, kT, va, bins, bouts, rscr, load_ans):
    """Transposed-scores causal attention (f32r QK^T and AV). Head-outer loop
    so each head's A2A chunk collective fires while later heads compute.

    P tile layout per (head, tau): full key blocks b < 4*tau at cols
    [b*512, (b+1)*512); the four diagonal blocks packed contiguously from
    DB = 4*tau*512 at offsets [0, 512, 896, 1152] with widths [512, 384,
    256, 128] (only q columns >= the block start are kept), so the 4
    diagonal exps merge into 2 ACT calls. pa/pb ping-pong on tau parity so
    the next iteration's exp can run while this iteration's AV drains."""
    DOFF = (0, 512, 896, 1152)
    with (
        tc.tile_pool(name="p2s", bufs=1, space="PSUM") as scp,
        tc.tile_pool(name="p2o", bufs=1, space="PSUM") as ovp,
        tc.tile_pool(name="pbuf", bufs=1) as pbp,
        tc.tile_pool(name="stg", bufs=1) as stp,
    ):
        for j in range(4):  # local q head
            for tau in range(NLC):
                t, hh = divmod(j, 2)
                po = 64 * hh  # partition base inside the pair tile
                nb = 4 * tau + 4
                DB = 4 * tau * 512
                par = tau % 2
                pcols = 5376 if par == 0 else 7424
                pa = pbp.tile([128, pcols], ATT, name="pa", tag=f"pa{par}")
                pb = pbp.tile([128, pcols], ATT, name="pb", tag=f"pb{par}")
                lcols = slice(tau * LC, (tau + 1) * LC)
                qa = qT[t][0][po : po + 64, lcols]
                qb = qT[t][1][po : po + 64, lcols]

                # full (unmasked) strips, two key-blocks per exp call
                for b0 in range(0, 4 * tau, 2):
                    for q, kTb, P in ((qa, kT[0], pa), (qb, kT[1], pb)):
                        sc = scp.tile(
                            [128, 1024], F32, name="sc", tag="sc", bufs=3
                        )
                        nc.tensor.matmul(
                            sc[:, 0:512],
                            kTb[po : po + 64, b0 * 128 : (b0 + 1) * 128],
                            q,
                        )
                        nc.tensor.matmul(
                            sc[:, 512:1024],
                            kTb[po : po + 64, (b0 + 1) * 128 : (b0 + 2) * 128],
                            q,
                        )
                        nc.scalar.activation(
                            P[:, b0 * 512 : (b0 + 2) * 512], sc[:, 0:1024], Exp
                        )
                # diagonal strips: only q columns >= block start are computed;
                # two merged exps (dj 0+1 -> 896 cols, dj 2+3 -> 384 cols)
                for b_, (kTb, P) in enumerate(((kT[0], pa), (kT[1], pb))):
                    for dpair in ((0, 1), (2, 3)):
                        sc = scp.tile(
                            [128, 1024], F32, name="sc", tag="sc", bufs=3
                        )
                        soff = 0
                        for dj in dpair:
                            beta = 4 * tau + dj
                            nv = 512 - dj * 128
                            qd = qT[t][b_][
                                po : po + 64,
                                tau * LC + dj * 128 : (tau + 1) * LC,
                            ]
                            nc.tensor.matmul(
                                sc[:, soff : soff + nv],
                                kTb[po : po + 64, beta * 128 : (beta + 1) * 128],
                                qd,
                            )
                            soff += nv
                        nc.scalar.activation(
                            P[:, DB + DOFF[dpair[0]] : DB + DOFF[dpair[0]] + soff],
                            sc[:, 0:soff],
                            Exp,
                        )
                    for dj in range(4):
                        dg = P[:, DB + DOFF[dj] : DB + DOFF[dj] + 128]
                        nc.gpsimd.affine_select(
                            out=dg,
                            in_=dg,
                            compare_op=mybir.AluOpType.is_ge,
                            fill=0.0,
                            base=0,
                            pattern=[[1, 128]],
                            channel_multiplier=-1,
                        )
                # AV (+denominator via the ones column of v_aug); diagonal
                # key blocks only accumulate into their valid q suffix
                oa = ovp.tile([128, 512], F32, name="oa", tag="oa")
                ob = ovp.tile([128, 512], F32, name="ob", tag="ob")
                for b in range(nb):
                    dj = b - 4 * tau
                    if dj < 0:
                        lo, pcol = 0, b * 512
                    else:
                        lo, pcol = dj * 128, DB + DOFF[dj]
                    st = dict(start=(b == 0), stop=(b == nb - 1))
                    nc.tensor.matmul(
                        oa[0:65, lo:512], va[:, b, 0:65],
                        pa[:, pcol : pcol + 512 - lo],
                        skip_group_check=True, **st,
                    )
                    nc.tensor.matmul(
                        ob[0:65, lo:512], va[:, b, 65:130],
                        pb[:, pcol : pcol + 512 - lo],
                        skip_group_check=True, **st,
                    )
                # normalize on the sender (softmax denominators come out of
                # the AV ones-column) so the A2A ships ready-to-use rows and
                # o_proj just DMA-loads them. DMA cannot read PSUM, so copy
                # through SBUF first (batch0 on Scalar, batch1 on Vector).
                # dest shard for (batch bb, l-block tau) is 4*bb + tau.
                st1a = stp.tile([64, 512], F32, name="st1a", tag="st1a")
                st1b = stp.tile([64, 512], F32, name="st1b", tag="st1b")
                nc.scalar.copy(st1a[:, :], oa[0:64, :])
                nc.vector.tensor_copy(st1b[:, :], ob[0:64, :])
                ds = stp.tile([128, 1024], F32, name="ds", tag="ds")
                nc.vector.tensor_copy(ds[64:65, 0:512], oa[64:65, :])
                nc.vector.tensor_copy(ds[64:65, 512:1024], ob[64:65, :])
                rrow = 4 * j + tau
                nc.sync.dma_start(rscr[rrow : rrow + 1, :], ds[64:65, :])
                dvb0 = stp.tile([64, 512], F32, name="dvb0", tag="dvb0")
                dvb1 = stp.tile([64, 512], F32, name="dvb1", tag="dvb1")
                nc.sync.dma_start(
                    dvb0[:, :],
                    rscr[rrow : rrow + 1, 0:512].partition_broadcast(64),
                )
                nc.sync.dma_start(
                    dvb1[:, :],
                    rscr[rrow : rrow + 1, 512:1024].partition_broadcast(64),
                )
                # reciprocal AFTER the broadcast: 64 partitions in parallel is
                # ~10x faster on DVE than one row
                nc.vector.reciprocal(dvb0[:, :], dvb0[:, :])
                nc.vector.reciprocal(dvb1[:, :], dvb1[:, :])
                stn0 = stp.tile([64, 512], BF16, name="stn0", tag="stn0")
                stn1 = stp.tile([64, 512], BF16, name="stn1", tag="stn1")
                nc.vector.tensor_mul(stn0[:, :], st1a[:, :], dvb0[:, :])
                nc.vector.tensor_mul(stn1[:, :], st1b[:, :], dvb1[:, :])
                for bb, half in ((0, stn0[:, :]), (1, stn1[:, :])):
                    sh = SH * (4 * bb + tau)
                    nc.sync.dma_start(bins[j][sh : sh + 64, :], half)
                if j == 3 and tau < 3:
                    # chunks 0-2 have landed by now; prefetch their o_proj
                    # ans tiles while head 3 finishes
                    load_ans(tau)
            # head j fully staged on every core -> overlap its A2A with the
            # attention compute of the remaining heads
            nc.gpsimd.collective_compute(
                "AllToAll",
                mybir.AluOpType.bypass,
                ins=[bins[j].opt()],
                outs=[bouts[j].opt()],
                replica_groups=[list(range(NCORES))],
            )


def _phase4_oproj(nc, tc, ans, load_ans, woT, wop, wo_pre, y):
    """o_proj for this core's 512 sequence rows against the full Wo, all in
    bf16 (the A2A chunks arrive pre-normalized as bf16). ans tile k holds
    heads (2k, 2k+1): chunks (0,1) fill even k, chunks (2,3) odd k. Chunks
    0-2's ans loads were prefetched during head 3 of attention. Two-pass
    contraction: the even half runs while chunk 3 is still in flight
    (spilling partials to SBUF), the odd half after it lands; DVE adds the
    halves."""
    with (
        tc.tile_pool(name="wo2", bufs=2) as wop2,
        tc.tile_pool(name="ysb", bufs=2) as yp,
        tc.tile_pool(name="esb", bufs=1) as esp,
        tc.tile_pool(name="p4y", bufs=4, space="PSUM") as eyp,
    ):
        # wo_e dc=1 right away (dc=0 was prefetched mid-attention)
        wo_pre[1] = _wo_load(nc, wop, woT, 1, 0)
        esbs = {}
        for dc in range(4):
            wo_e = wo_pre[dc] if dc in wo_pre else _wo_load(nc, wop, woT, dc, 0)
            for m in range(4):
                yps = eyp.tile([128, 512], F32, name="ypsE", tag="ypsE")
                for i, k in enumerate(range(0, 16, 2)):
                    nc.tensor.matmul(
                        yps[:, :],
                        ans[k][:, m * 128 : (m + 1) * 128],
                        wo_e[:, i * 512 : (i + 1) * 512],
                        start=(i == 0), stop=(i == 7),
                    )
                esb = esp.tile([128, 512], F32, name=f"esb{dc}{m}")
                nc.scalar.copy(esb[:, :], yps[:, :])
                esbs[(dc, m)] = esb
        # odd-half Wo prefetches BEFORE the chunk-3 ans DMAs: those wait on
        # cc3 and would block the queue behind them
        wo_os = {dc: _wo_load(nc, wop2, woT, dc, 1) for dc in range(2)}
        load_ans(3)
        for dc in range(4):
            wo_o = wo_os[dc] if dc in wo_os else _wo_load(nc, wop2, woT, dc, 1)
            for m in range(4):
                yps = eyp.tile([128, 512], F32, name="ypsO", tag="ypsO")
                for i, k in enumerate(range(1, 16, 2)):
                    nc.tensor.matmul(
                        yps[:, :],
                        ans[k][:, m * 128 : (m + 1) * 128],
                        wo_o[:, i * 512 : (i + 1) * 512],
                        start=(i == 0), stop=(i == 7),
                    )
                ysb = yp.tile([128, 512], F32, name="ysb", tag="ysb")
                nc.vector.tensor_add(ysb[:, :], yps[:, :], esbs[(dc, m)][:, :])
                nc.sync.dma_start(
                    y[m * 128 : (m + 1) * 128, dc * 512 : (dc + 1) * 512], ysb[:, :]
                )


def _get_nc():
    if "nc" not in _CACHE:
        _CACHE["nc"] = _build_nc()
    return _CACHE["nc"]


LAST_EXEC_NS = None


def kernel(x, Wq, Wk, Wv, Wo):
    global LAST_EXEC_NS
    x = np.asarray(x, dtype=np.float32)
    Wq = np.asarray(Wq, dtype=np.float32)
    Wk = np.asarray(Wk, dtype=np.float32)
    Wv = np.asarray(Wv, dtype=np.float32)
    Wo = np.asarray(Wo, dtype=np.float32)

    xT0 = np.ascontiguousarray(x[0].T)
    xT1 = np.ascontiguousarray(x[1].T)
    woT = np.ascontiguousarray(Wo.T)

    in_maps = []
    for c in range(NCORES):
        wqT_c = np.ascontiguousarray((SCALE * Wq[QF * c : QF * (c + 1), :]).T)
        wkT_c = np.ascontiguousarray(Wk[DH * c : DH * (c + 1), :].T)
        wvT_c = np.ascontiguousarray(Wv[DH * c : DH * (c + 1), :].T)
        in_maps.append(
            {
                "xT0": xT0,
                "xT1": xT1,
                "wqT": wqT_c,
                "wkT": wkT_c,
                "wvT": wvT_c,
                "woT": woT,
            }
        )

    nc = _get_nc()
    res = run_bass_kernel_spmd(nc, in_maps, core_ids=list(range(NCORES)))
    LAST_EXEC_NS = getattr(res, "exec_time_ns", None)

    out = np.empty((B, L, D), dtype=np.float32)
    for c in range(NCORES):
        b, g = divmod(c, 4)
        out[b, 512 * g : 512 * (g + 1), :] = res.results[c]["y"]
    return out

